# revision 16
# baseline (speedup 1.0000x reference)
"""Criss-cross attention block kernel for Trainium2 (Bass/Tile), 8-core data parallel.

Problem (hardcoded shapes): x [8, 96, 96, 512] fp32.
  q = x@Wq+bq, k = x@Wk+bk (c=64), v = x@Wv+bv (C=512)
  per pixel (h,w): softmax over [scores vs column (diag masked), scores vs row],
  y = gamma * (att_v + att_h) + x.

Sharding: pure data parallel, one batch image per NeuronCore (B=8 = n_cores).

Per-core algorithm (v2 — all repartitioning via HBM with contiguous patterns):
  Phase 1: stream x in 512-pixel tiles (alternating HWDGE queues); PE-transpose
           to xT (C on partitions); project qT,kT (fp16, SBUF resident
           [64, 9216]) and v (bf16 -> HBM scratch, pixel-major).
  Phase 2 (per column w): S^T = K_w @ Q_w^T; E = exp(S^T) (no max subtraction:
           |s| <= ~60 fits fp32/bf16); zero diagonal; yv_w = E^T.T @ V_w and
           Zv_w = E^T.T @ ones. yv staged bf16 and written to a pixel-major
           HBM scratch (8-column blocks -> 8KB-contiguous runs, NO slow
           SBUF->SBUF partition-crossing DMAs).
  Phase 3 (per row r): Eh scores; yp = Eh^T.T @ V_row + I.T @ yv_row (both row
           tiles are contiguous HBM reads); y = (yp * gamma/(Zv+Zh)) + x via
           one fused DVE op per row; row-major y store.
"""

import os

import numpy as np
import ml_dtypes

import concourse.bass as bass
import concourse.mybir as mybir
import concourse.tile as tile
from concourse import bacc
from concourse.bass import ts, ds
from concourse.masks import make_identity
from concourse.bass_utils import run_bass_kernel_spmd

F32 = mybir.dt.float32
F32R = mybir.dt.float32r
BF16 = mybir.dt.bfloat16
FP16 = mybir.dt.float16

H = 96
W = 96
C = 512
CQK = 64
NPIX = H * W  # 9216
N_CORES = 8

# phase-1 pixel tiling: PT pixels per tile, PS = PT//128 subchunks
PT = 512
PS = PT // 128
NT = NPIX // PT  # 18
KC = C // 128  # 4 contraction chunks
# phase-2/3 blocking (columns/rows per block)
WB = 8
RB = 4


def build_nc() -> bass.Bass:
    nc = bacc.Bacc(
        "TRN2", target_bir_lowering=False, debug=False, num_devices=N_CORES
    )

    x = nc.dram_tensor("x", [NPIX, C], F32, kind="ExternalInput")[:]
    Wq = nc.dram_tensor("Wq", [C, CQK], F32, kind="ExternalInput")[:]
    bq = nc.dram_tensor("bq", [CQK], F32, kind="ExternalInput")[:]
    Wk = nc.dram_tensor("Wk", [C, CQK], F32, kind="ExternalInput")[:]
    bk = nc.dram_tensor("bk", [CQK], F32, kind="ExternalInput")[:]
    Wv = nc.dram_tensor("Wv", [C, C], F32, kind="ExternalInput")[:]
    bv = nc.dram_tensor("bv", [C], F32, kind="ExternalInput")[:]
    gamma = nc.dram_tensor("gamma", [1, 1], F32, kind="ExternalInput")[:]
    # y is produced bf16 AND column-major (pixel order (w,h)): the phase-3 row
    # stores then write 4KB-contiguous runs per partition instead of 4x1KB.
    # The host upconverts + untransposes.
    y = nc.dram_tensor("y", [NPIX, C], BF16, kind="ExternalOutput")[:]

    with tile.TileContext(nc) as tc:
        _body(nc, tc, x, Wq, bq, Wk, bk, Wv, bv, gamma, y)
    nc.compile()
    return nc


def _body(nc, tc, x, Wq, bq, Wk, bk, Wv, bv, gamma, y):
    mult = mybir.AluOpType.mult
    add = mybir.AluOpType.add

    with (
        tc.tile_pool(name="singles", bufs=1) as singles,
        tc.tile_pool(name="dram", bufs=1, space="DRAM") as dram,
    ):
        # ---- constants / weights resident in SBUF ----
        identity128 = singles.tile([128, 128], F32)
        make_identity(nc, identity128)
        identity_h = singles.tile([128, 128], FP16)
        make_identity(nc, identity_h)
        id96b = singles.tile([96, 96], BF16)
        make_identity(nc, id96b)
        # anti-diagonal mask: 1 everywhere, 0 on diag
        antidiag = singles.tile([96, 96], BF16)
        nc.vector.memset(antidiag, 1.0)
        nc.gpsimd.affine_select(
            out=antidiag,
            in_=antidiag,
            compare_op=mybir.AluOpType.not_equal,
            fill=0.0,
            base=0,
            pattern=[[-1, 96]],
            channel_multiplier=1,
        )
        ones96 = singles.tile([96, 1], BF16)
        nc.vector.memset(ones96, 1.0)

        # weights: DMA fp32 staging, round to fp16 tiles (separate memlocs).
        # Wq and Wk are stacked into one [128, KC, 128] stationary so q and k
        # project in a single matmul group per contraction chunk.
        wqk_sb = singles.tile([128, KC, 2 * CQK], FP16)
        wv_sb = singles.tile([128, KC, C], FP16)
        with tc.tile_pool(name="wstage", bufs=1) as wstage:
            wq_f32 = wstage.tile([128, KC, CQK], F32)
            nc.sync.dma_start(out=wq_f32, in_=Wq.rearrange("(o p) d -> p o d", p=128))
            wk_f32 = wstage.tile([128, KC, CQK], F32)
            nc.sync.dma_start(out=wk_f32, in_=Wk.rearrange("(o p) d -> p o d", p=128))
            wv_f32 = wstage.tile([128, KC, C], F32)
            nc.sync.dma_start(out=wv_f32, in_=Wv.rearrange("(o p) d -> p o d", p=128))
            nc.vector.tensor_copy(out=wqk_sb[:, :, :CQK], in_=wq_f32)
            nc.vector.tensor_copy(out=wqk_sb[:, :, CQK:], in_=wk_f32)
            nc.vector.tensor_copy(out=wv_sb, in_=wv_f32)

        bq_sb = singles.tile([CQK, 1], F32)
        nc.sync.dma_start(out=bq_sb, in_=bq[:, None])
        bk_sb = singles.tile([CQK, 1], F32)
        nc.sync.dma_start(out=bk_sb, in_=bk[:, None])
        bv_sb = singles.tile([128, C], F32)
        nc.sync.dma_start(
            out=bv_sb,
            in_=bass.AP(tensor=bv.tensor, offset=bv.offset, ap=[[0, 128], *bv.ap]),
        )
        gamma_sb = singles.tile([128, 1], F32)
        nc.sync.dma_start(
            out=gamma_sb,
            in_=bass.AP(
                tensor=gamma.tensor, offset=gamma.offset, ap=[[0, 128], [1, 1]]
            ),
        )

        # ---- persistent per-image state ----
        qT_sb = singles.tile([CQK, NPIX], FP16)
        kT_sb = singles.tile([CQK, NPIX], FP16)
        zv_img = singles.tile([96, W], F32)  # [h, w]
        zv_T = singles.tile([96, 96], F32)  # [w, h]
        rzg_all = singles.tile([96, H], F32)  # gamma/(Zv+Zh), [u, r]
        v_hbm = dram.tile([NPIX, C], BF16)
        yv_hbm = dram.tile([NPIX, C], BF16)  # unnormalized att_v, pixel-major

        qT_v = qT_sb.rearrange("d (h w) -> d h w", w=W)
        kT_v = kT_sb.rearrange("d (h w) -> d h w", w=W)
        v_img_view = v_hbm.rearrange("(h w) c -> h w c", w=W)
        yv_img_view = yv_hbm.rearrange("(h w) c -> h w c", w=W)

        # ================= Phase 1: projections =================
        x_r = x.rearrange("(t s p) c -> t p s c", s=PS, p=128)
        vout_r = v_hbm.rearrange("(t s p) c -> t p s c", s=PS, p=128)
        with (
            tc.tile_pool(name="p1", bufs=2) as p1,
            tc.tile_pool(name="psA", bufs=4, space="PSUM") as psA,
            tc.tile_pool(name="psB", bufs=3, space="PSUM") as psB,
        ):
            xT_ts = {}

            def p1_stage_a(t):
                # load f32 (HWDGE queues can't cast), cast to fp16 on the
                # Scalar engine, then 1-cycle/row fp16 PE transposes.
                x_t = p1.tile([128, PS, C], F32, name="x_t", bufs=3)
                eng = [nc.sync, nc.scalar][t % 2]
                eng.dma_start(out=x_t, in_=x_r[t])
                x_h = p1.tile([128, PS, C], FP16, name="x_h", bufs=3)
                nc.scalar.copy(out=x_h, in_=x_t)
                xT_t = p1.tile([128, KC, PT], FP16, name="xT_t", bufs=3)
                for s in range(PS):
                    tp_ps = psA.tile(
                        [128, KC, 128], FP16, name="tp_ps", tag="tp", bufs=2
                    )
                    for cc in range(KC):
                        nc.tensor.transpose(
                            tp_ps[:, cc, :], x_h[:, s, ts(cc, 128)], identity_h
                        )
                    if (t * PS + s) % 3 == 2:
                        nc.scalar.copy(out=xT_t[:, :, ts(s, 128)], in_=tp_ps)
                    else:
                        nc.vector.tensor_copy(out=xT_t[:, :, ts(s, 128)], in_=tp_ps)
                xT_ts[t] = xT_t

            def p1_stage_b(t):
                xT_t = xT_ts.pop(t)
                qkp = psB.tile([2 * CQK, PT], F32, name="qkp", tag="qk", bufs=3)
                for cc in range(KC):
                    nc.tensor.matmul(
                        qkp,
                        lhsT=wqk_sb[:, cc, :],
                        rhs=xT_t[:, cc, :],
                        start=(cc == 0),
                        stop=(cc == KC - 1),
                    )
                nc.scalar.activation(
                    out=qT_sb[:, ts(t, PT)],
                    in_=qkp[:CQK, :],
                    func=mybir.ActivationFunctionType.Identity,
                    bias=bq_sb,
                    scale=1.0,
                )
                nc.scalar.activation(
                    out=kT_sb[:, ts(t, PT)],
                    in_=qkp[CQK:, :],
                    func=mybir.ActivationFunctionType.Identity,
                    bias=bk_sb,
                    scale=1.0,
                )
                # v projection: out [128 pix, 512]
                v_st = p1.tile([128, PS, C], BF16, name="v_st")
                for m in range(PS):
                    vp = psA.tile([128, C], F32, name="vp", tag="mm", bufs=3)
                    for cc in range(KC):
                        nc.tensor.matmul(
                            vp,
                            lhsT=xT_t[:, cc, ts(m, 128)],
                            rhs=wv_sb[:, cc, :],
                            start=(cc == 0),
                            stop=(cc == KC - 1),
                        )
                    nc.vector.tensor_add(out=v_st[:, m, :], in0=vp, in1=bv_sb)
                nc.gpsimd.dma_start(out=vout_r[t], in_=v_st)

            p1_stage_a(0)
            for t in range(NT):
                if t + 1 < NT:
                    p1_stage_a(t + 1)
                p1_stage_b(t)

        # ================= Phase 2: vertical (columns) =================
        # Stage (a): scores + exp + mask into a double-buffered E block.
        # Stage (b): yv / Zv matmuls; yv staged bf16 and DMA'd to pixel-major
        # HBM scratch in 8-column blocks (8KB-contiguous runs per h).
        NB2 = W // WB
        with (
            tc.tile_pool(name="p2", bufs=3) as p2,
            tc.tile_pool(name="psC", bufs=4, space="PSUM") as psC,
            tc.tile_pool(name="psD", bufs=2, space="PSUM") as psD,
        ):
            e_blks = {}

            def p2_scores(wb):
                e_blk = p2.tile([96, WB, 96], BF16, name="e_blk", bufs=3)
                for half in range(WB // 4):
                    sp = psD.tile([96, 4, 96], F32, name="sp", tag="sp", bufs=2)
                    for i in range(4):
                        w = wb * WB + half * 4 + i
                        nc.tensor.matmul(
                            sp[:, i, :], lhsT=kT_v[:, :, w], rhs=qT_v[:, :, w]
                        )
                    nc.scalar.activation(
                        out=e_blk[:, half * 4 : half * 4 + 4, :],
                        in_=sp,
                        func=mybir.ActivationFunctionType.Exp,
                    )
                nc.vector.tensor_mul(
                    out=e_blk,
                    in0=e_blk,
                    in1=antidiag[:, None, :].to_broadcast((96, WB, 96)),
                )
                e_blks[wb] = e_blk

            def p2_consume(wb):
                e_blk = e_blks.pop(wb)
                vcol = p2.tile([96, WB, C], BF16, name="vcol", bufs=3)
                nc.sync.dma_start(out=vcol, in_=v_img_view[:, ts(wb, WB), :])
                yv_st = p2.tile([96, WB, C], BF16, name="yv_st", bufs=2)
                zp = psD.tile([96, WB], F32, name="zp", tag="zp", bufs=2)
                for wi in range(WB):
                    w = wb * WB + wi
                    yvp = psC.tile([96, C], F32, name="yvp", tag="mm", bufs=4)
                    nc.tensor.matmul(yvp, lhsT=e_blk[:, wi, :], rhs=vcol[:, wi, :])
                    nc.tensor.matmul(
                        zp[:, wi : wi + 1], lhsT=e_blk[:, wi, :], rhs=ones96
                    )
                    if wi % 2 == 0:
                        nc.scalar.copy(out=yv_st[:, wi, :], in_=yvp)
                    else:
                        nc.vector.tensor_copy(out=yv_st[:, wi, :], in_=yvp)
                nc.vector.tensor_copy(out=zv_img[:, ts(wb, WB)], in_=zp)
                eng = [nc.scalar, nc.gpsimd][wb % 2]
                eng.dma_start(out=yv_img_view[:, ts(wb, WB), :], in_=yv_st)

            p2_scores(0)
            p2_scores(1)
            for wb in range(NB2):
                if wb + 2 < NB2:
                    p2_scores(wb + 2)
                p2_consume(wb)

            # transpose Zv image once: [h, w] -> [w, h]
            ztp = psD.tile([96, 96], F32, name="ztp", tag="sp", bufs=2)
            nc.tensor.transpose(ztp, zv_img, identity128[:96, :96])
            nc.vector.tensor_copy(out=zv_T, in_=ztp)

        # ================= Phase 3: horizontal (rows) + combine =================
        x_rows = x.rearrange("(rb r u) c -> rb u r c", r=RB, u=W)
        # y is column-major: pixel (r, u) lives at row u*H + r
        y_rows = y.rearrange("(u rb r) c -> rb u r c", r=RB, rb=H // RB)
        v_rows = v_hbm.rearrange("(rb r u) c -> rb u r c", r=RB, u=W)
        yv_rows = yv_hbm.rearrange("(rb r u) c -> rb u r c", r=RB, u=W)
        with (
            tc.tile_pool(name="p3", bufs=3) as p3,
            tc.tile_pool(name="psE", bufs=3, space="PSUM") as psE,
            tc.tile_pool(name="psF", bufs=2, space="PSUM") as psF,
        ):
            NB3 = H // RB
            e3_blks = {}

            def p3_scores(rb):
                sp3 = psF.tile([96, RB, 96], F32, name="sp3", tag="sp")
                for ri in range(RB):
                    r = rb * RB + ri
                    nc.tensor.matmul(
                        sp3[:, ri, :], lhsT=kT_v[:, r, :], rhs=qT_v[:, r, :]
                    )
                e3_blk = p3.tile([96, RB, 96], BF16, name="e3_blk", bufs=3)
                nc.scalar.activation(
                    out=e3_blk, in_=sp3, func=mybir.ActivationFunctionType.Exp
                )
                e3_blks[rb] = e3_blk

            def p3_consume(rb):
                e3_blk = e3_blks.pop(rb)
                # spread ~47 MB of row traffic evenly over all three queues
                qs = [nc.sync, nc.scalar, nc.gpsimd]
                vrow = p3.tile([96, RB, C], BF16, name="vrow", bufs=3)
                qs[(rb + 1) % 3].dma_start(out=vrow, in_=v_rows[rb])
                yvrow = p3.tile([96, RB, C], BF16, name="yvrow", bufs=3)
                qs[(rb + 2) % 3].dma_start(out=yvrow, in_=yv_rows[rb])
                xrow = p3.tile([96, RB, C], F32, name="xrow", bufs=3)
                qs[rb % 3].dma_start(out=xrow, in_=x_rows[rb])
                # Z for the whole block, then batched gamma/(Zv+Zh)
                zp3 = psF.tile([96, RB], F32, name="zp3", tag="zp")
                for ri in range(RB):
                    nc.tensor.matmul(
                        zp3[:, ri : ri + 1], lhsT=e3_blk[:, ri, :], rhs=ones96
                    )
                rzg_blk = rzg_all[:, ts(rb, RB)]
                nc.vector.tensor_add(
                    out=rzg_blk, in0=zp3, in1=zv_T[:, ts(rb, RB)]
                )
                nc.vector.reciprocal(out=rzg_blk, in_=rzg_blk)
                nc.vector.tensor_scalar_mul(
                    out=rzg_blk, in0=rzg_blk, scalar1=gamma_sb[:96, :]
                )
                yrow_st = p3.tile([96, RB, C], BF16, name="yrow_st", bufs=3)
                for ri in range(RB):
                    r = rb * RB + ri
                    yp = psE.tile([96, C], F32, name="yp", tag="mm")
                    nc.tensor.matmul(yp, lhsT=e3_blk[:, ri, :], rhs=vrow[:, ri, :],
                                     start=True, stop=False)
                    nc.tensor.matmul(yp, lhsT=id96b, rhs=yvrow[:, ri, :],
                                     start=False, stop=True)
                    nc.vector.scalar_tensor_tensor(
                        out=yrow_st[:, ri, :],
                        in0=yp,
                        scalar=rzg_all[:, ds(r, 1)],
                        in1=xrow[:, ri, :],
                        op0=mult,
                        op1=add,
                    )
                qs[(rb + 1) % 3].dma_start(out=y_rows[rb], in_=yrow_st)

            p3_scores(0)
            for rb in range(NB3):
                if rb + 1 < NB3:
                    p3_scores(rb + 1)
                p3_consume(rb)


_NC_CACHE = None


def _get_nc():
    global _NC_CACHE
    if _NC_CACHE is None:
        _NC_CACHE = build_nc()
    return _NC_CACHE


def run(inputs: dict, trace: bool = False):
    """Run on 8 cores; returns (full_output [8,96,96,512] f32, BassKernelResults)."""
    x = np.ascontiguousarray(np.asarray(inputs["x"], dtype=np.float32))
    B = x.shape[0]
    assert x.shape == (N_CORES, H, W, C), x.shape
    common = {
        "Wq": np.ascontiguousarray(np.asarray(inputs["Wq"], np.float32)),
        "bq": np.ascontiguousarray(np.asarray(inputs["bq"], np.float32)),
        "Wk": np.ascontiguousarray(np.asarray(inputs["Wk"], np.float32)),
        "bk": np.ascontiguousarray(np.asarray(inputs["bk"], np.float32)),
        "Wv": np.ascontiguousarray(np.asarray(inputs["Wv"], np.float32)),
        "bv": np.ascontiguousarray(np.asarray(inputs["bv"], np.float32)),
        "gamma": np.asarray(inputs["gamma"], np.float32).reshape(1, 1).copy(),
    }
    in_maps = [
        {"x": x[b].reshape(NPIX, C), **common} for b in range(B)
    ]
    nc = _get_nc()
    res = run_bass_kernel_spmd(
        nc, in_maps, core_ids=list(range(N_CORES)), trace=trace
    )
    out = np.stack(
        [
            np.asarray(res.results[b]["y"], dtype=np.float32)
            .reshape(W, H, C)
            .transpose(1, 0, 2)
            for b in range(B)
        ],
        axis=0,
    )
    return out, res


def kernel(**inputs) -> np.ndarray:
    out, _ = run(inputs, trace=False)
    return out


if __name__ == "__main__":
    nc = build_nc()
    print("built ok")


# revision 19
# speedup vs baseline: 1.0275x; 1.0275x over previous
"""Criss-cross attention block kernel for Trainium2 (Bass/Tile), 8-core data parallel.

Problem (hardcoded shapes): x [8, 96, 96, 512] fp32.
  q = x@Wq+bq, k = x@Wk+bk (c=64), v = x@Wv+bv (C=512)
  per pixel (h,w): softmax over [scores vs column (diag masked), scores vs row],
  y = gamma * (att_v + att_h) + x.

Sharding: pure data parallel, one batch image per NeuronCore (B=8 = n_cores).

Per-core algorithm (v2 — all repartitioning via HBM with contiguous patterns):
  Phase 1: stream x in 512-pixel tiles (alternating HWDGE queues); PE-transpose
           to xT (C on partitions); project qT,kT (fp16, SBUF resident
           [64, 9216]) and v (bf16 -> HBM scratch, pixel-major).
  Phase 2 (per column w): S^T = K_w @ Q_w^T; E = exp(S^T) (no max subtraction:
           |s| <= ~60 fits fp32/bf16); zero diagonal; yv_w = E^T.T @ V_w and
           Zv_w = E^T.T @ ones. yv staged bf16 and written to a pixel-major
           HBM scratch (8-column blocks -> 8KB-contiguous runs, NO slow
           SBUF->SBUF partition-crossing DMAs).
  Phase 3 (per row r): Eh scores; yp = Eh^T.T @ V_row + I.T @ yv_row (both row
           tiles are contiguous HBM reads); y = (yp * gamma/(Zv+Zh)) + x via
           one fused DVE op per row; row-major y store.
"""

import os

import numpy as np
import ml_dtypes

import concourse.bass as bass
import concourse.mybir as mybir
import concourse.tile as tile
from concourse import bacc
from concourse.bass import ts, ds
from concourse.masks import make_identity
from concourse.bass_utils import run_bass_kernel_spmd

F32 = mybir.dt.float32
F32R = mybir.dt.float32r
BF16 = mybir.dt.bfloat16
FP16 = mybir.dt.float16

H = 96
W = 96
C = 512
CQK = 64
NPIX = H * W  # 9216
N_CORES = 8

# phase-1 pixel tiling: PT pixels per tile, PS = PT//128 subchunks
PT = 512
PS = PT // 128
NT = NPIX // PT  # 18
KC = C // 128  # 4 contraction chunks
# phase-2/3 blocking (columns/rows per block)
WB = 8
RB = 4


def build_nc() -> bass.Bass:
    nc = bacc.Bacc(
        "TRN2", target_bir_lowering=False, debug=False, num_devices=N_CORES
    )

    x = nc.dram_tensor("x", [NPIX, C], F32, kind="ExternalInput")[:]
    Wq = nc.dram_tensor("Wq", [C, CQK], F32, kind="ExternalInput")[:]
    bq = nc.dram_tensor("bq", [CQK], F32, kind="ExternalInput")[:]
    Wk = nc.dram_tensor("Wk", [C, CQK], F32, kind="ExternalInput")[:]
    bk = nc.dram_tensor("bk", [CQK], F32, kind="ExternalInput")[:]
    Wv = nc.dram_tensor("Wv", [C, C], F32, kind="ExternalInput")[:]
    bv = nc.dram_tensor("bv", [C], F32, kind="ExternalInput")[:]
    gamma = nc.dram_tensor("gamma", [1, 1], F32, kind="ExternalInput")[:]
    # y is produced bf16 AND column-major (pixel order (w,h)): the phase-3 row
    # stores then write 4KB-contiguous runs per partition instead of 4x1KB.
    # The host upconverts + untransposes.
    y = nc.dram_tensor("y", [NPIX, C], BF16, kind="ExternalOutput")[:]

    with tile.TileContext(nc) as tc:
        _body(nc, tc, x, Wq, bq, Wk, bk, Wv, bv, gamma, y)
    nc.compile()
    return nc


def _body(nc, tc, x, Wq, bq, Wk, bk, Wv, bv, gamma, y):
    mult = mybir.AluOpType.mult
    add = mybir.AluOpType.add

    with (
        tc.tile_pool(name="singles", bufs=1) as singles,
        tc.tile_pool(name="dram", bufs=1, space="DRAM") as dram,
    ):
        # ---- constants / weights resident in SBUF ----
        identity128 = singles.tile([128, 128], F32)
        make_identity(nc, identity128)
        identity_h = singles.tile([128, 128], FP16)
        make_identity(nc, identity_h)
        id96b = singles.tile([96, 96], BF16)
        make_identity(nc, id96b)
        # anti-diagonal mask: 1 everywhere, 0 on diag
        antidiag = singles.tile([96, 96], BF16)
        nc.vector.memset(antidiag, 1.0)
        nc.gpsimd.affine_select(
            out=antidiag,
            in_=antidiag,
            compare_op=mybir.AluOpType.not_equal,
            fill=0.0,
            base=0,
            pattern=[[-1, 96]],
            channel_multiplier=1,
        )
        ones96 = singles.tile([96, 1], BF16)
        nc.vector.memset(ones96, 1.0)

        # weights: DMA fp32 staging, round to fp16 tiles (separate memlocs).
        # Wq and Wk are stacked into one [128, KC, 128] stationary so q and k
        # project in a single matmul group per contraction chunk.
        wqk_sb = singles.tile([128, KC, 2 * CQK], FP16)
        wv_sb = singles.tile([128, KC, C], FP16)
        with tc.tile_pool(name="wstage", bufs=1) as wstage:
            wq_f32 = wstage.tile([128, KC, CQK], F32)
            nc.sync.dma_start(out=wq_f32, in_=Wq.rearrange("(o p) d -> p o d", p=128))
            wk_f32 = wstage.tile([128, KC, CQK], F32)
            nc.sync.dma_start(out=wk_f32, in_=Wk.rearrange("(o p) d -> p o d", p=128))
            wv_f32 = wstage.tile([128, KC, C], F32)
            nc.sync.dma_start(out=wv_f32, in_=Wv.rearrange("(o p) d -> p o d", p=128))
            nc.vector.tensor_copy(out=wqk_sb[:, :, :CQK], in_=wq_f32)
            nc.vector.tensor_copy(out=wqk_sb[:, :, CQK:], in_=wk_f32)
            nc.vector.tensor_copy(out=wv_sb, in_=wv_f32)

        bq_sb = singles.tile([CQK, 1], F32)
        nc.sync.dma_start(out=bq_sb, in_=bq[:, None])
        bk_sb = singles.tile([CQK, 1], F32)
        nc.sync.dma_start(out=bk_sb, in_=bk[:, None])
        bv_sb = singles.tile([128, C], F32)
        nc.sync.dma_start(
            out=bv_sb,
            in_=bass.AP(tensor=bv.tensor, offset=bv.offset, ap=[[0, 128], *bv.ap]),
        )
        gamma_sb = singles.tile([128, 1], F32)
        nc.sync.dma_start(
            out=gamma_sb,
            in_=bass.AP(
                tensor=gamma.tensor, offset=gamma.offset, ap=[[0, 128], [1, 1]]
            ),
        )

        # ---- persistent per-image state ----
        qT_sb = singles.tile([CQK, NPIX], FP16)
        kT_sb = singles.tile([CQK, NPIX], FP16)
        zv_img = singles.tile([96, W], F32)  # [h, w]
        zv_T = singles.tile([96, 96], F32)  # [w, h]
        rzg_all = singles.tile([96, H], F32)  # gamma/(Zv+Zh), [u, r]
        v_hbm = dram.tile([NPIX, C], BF16)
        yv_hbm = dram.tile([NPIX, C], BF16)  # unnormalized att_v, pixel-major

        qT_v = qT_sb.rearrange("d (h w) -> d h w", w=W)
        kT_v = kT_sb.rearrange("d (h w) -> d h w", w=W)
        v_img_view = v_hbm.rearrange("(h w) c -> h w c", w=W)
        yv_img_view = yv_hbm.rearrange("(h w) c -> h w c", w=W)

        # ================= Phase 1: projections =================
        x_r = x.rearrange("(t s p) c -> t p s c", s=PS, p=128)
        vout_r = v_hbm.rearrange("(t s p) c -> t p s c", s=PS, p=128)
        with (
            tc.tile_pool(name="p1", bufs=2) as p1,
            tc.tile_pool(name="psA", bufs=4, space="PSUM") as psA,
            tc.tile_pool(name="psB", bufs=3, space="PSUM") as psB,
        ):
            xT_ts = {}

            def p1_stage_a(t):
                # load f32 (HWDGE queues can't cast), cast to fp16 on the
                # Scalar engine, then 1-cycle/row fp16 PE transposes.
                x_t = p1.tile([128, PS, C], F32, name="x_t", bufs=3)
                eng = [nc.sync, nc.scalar][t % 2]
                eng.dma_start(out=x_t, in_=x_r[t])
                x_h = p1.tile([128, PS, C], FP16, name="x_h", bufs=3)
                nc.scalar.copy(out=x_h, in_=x_t)
                xT_t = p1.tile([128, KC, PT], FP16, name="xT_t", bufs=3)
                for s in range(PS):
                    tp_ps = psA.tile(
                        [128, KC, 128], FP16, name="tp_ps", tag="tp", bufs=2
                    )
                    for cc in range(KC):
                        nc.tensor.transpose(
                            tp_ps[:, cc, :], x_h[:, s, ts(cc, 128)], identity_h
                        )
                    if (t * PS + s) % 3 == 2:
                        nc.scalar.copy(out=xT_t[:, :, ts(s, 128)], in_=tp_ps)
                    else:
                        nc.vector.tensor_copy(out=xT_t[:, :, ts(s, 128)], in_=tp_ps)
                xT_ts[t] = xT_t

            def p1_stage_b(t):
                xT_t = xT_ts.pop(t)
                qkp = psB.tile([2 * CQK, PT], F32, name="qkp", tag="qk", bufs=3)
                for cc in range(KC):
                    nc.tensor.matmul(
                        qkp,
                        lhsT=wqk_sb[:, cc, :],
                        rhs=xT_t[:, cc, :],
                        start=(cc == 0),
                        stop=(cc == KC - 1),
                    )
                nc.scalar.activation(
                    out=qT_sb[:, ts(t, PT)],
                    in_=qkp[:CQK, :],
                    func=mybir.ActivationFunctionType.Identity,
                    bias=bq_sb,
                    scale=1.0,
                )
                nc.scalar.activation(
                    out=kT_sb[:, ts(t, PT)],
                    in_=qkp[CQK:, :],
                    func=mybir.ActivationFunctionType.Identity,
                    bias=bk_sb,
                    scale=1.0,
                )
                # v projection: out [128 pix, 512]
                v_st = p1.tile([128, PS, C], BF16, name="v_st")
                for m in range(PS):
                    vp = psA.tile([128, C], F32, name="vp", tag="mm", bufs=3)
                    for cc in range(KC):
                        nc.tensor.matmul(
                            vp,
                            lhsT=xT_t[:, cc, ts(m, 128)],
                            rhs=wv_sb[:, cc, :],
                            start=(cc == 0),
                            stop=(cc == KC - 1),
                        )
                    nc.vector.tensor_add(out=v_st[:, m, :], in0=vp, in1=bv_sb)
                nc.gpsimd.dma_start(out=vout_r[t], in_=v_st)

            p1_stage_a(0)
            for t in range(NT):
                if t + 1 < NT:
                    p1_stage_a(t + 1)
                p1_stage_b(t)

        # ================= Phase 2: vertical (columns) =================
        # Stage (a): scores + exp + mask into a double-buffered E block.
        # Stage (b): yv / Zv matmuls; yv staged bf16 and DMA'd to pixel-major
        # HBM scratch in 8-column blocks (8KB-contiguous runs per h).
        NB2 = W // WB
        with (
            tc.tile_pool(name="p2", bufs=3) as p2,
            tc.tile_pool(name="psC", bufs=4, space="PSUM") as psC,
            tc.tile_pool(name="psD", bufs=2, space="PSUM") as psD,
        ):
            e_blks = {}

            def p2_scores(wb):
                e_blk = p2.tile([96, WB, 96], BF16, name="e_blk", bufs=3)
                for half in range(WB // 4):
                    sp = psD.tile([96, 4, 96], F32, name="sp", tag="sp", bufs=2)
                    for i in range(4):
                        w = wb * WB + half * 4 + i
                        nc.tensor.matmul(
                            sp[:, i, :], lhsT=kT_v[:, :, w], rhs=qT_v[:, :, w]
                        )
                    nc.scalar.activation(
                        out=e_blk[:, half * 4 : half * 4 + 4, :],
                        in_=sp,
                        func=mybir.ActivationFunctionType.Exp,
                    )
                nc.vector.tensor_mul(
                    out=e_blk,
                    in0=e_blk,
                    in1=antidiag[:, None, :].to_broadcast((96, WB, 96)),
                )
                e_blks[wb] = e_blk

            def p2_consume(wb):
                e_blk = e_blks.pop(wb)
                vcol = p2.tile([96, WB, C], BF16, name="vcol", bufs=6)
                nc.sync.dma_start(out=vcol, in_=v_img_view[:, ts(wb, WB), :])
                yv_st = p2.tile([96, WB, C], BF16, name="yv_st", bufs=2)
                zp = psD.tile([96, WB], F32, name="zp", tag="zp", bufs=2)
                for wi in range(WB):
                    w = wb * WB + wi
                    yvp = psC.tile([96, C], F32, name="yvp", tag="mm", bufs=4)
                    nc.tensor.matmul(yvp, lhsT=e_blk[:, wi, :], rhs=vcol[:, wi, :])
                    nc.tensor.matmul(
                        zp[:, wi : wi + 1], lhsT=e_blk[:, wi, :], rhs=ones96
                    )
                    if wi % 2 == 0:
                        nc.scalar.copy(out=yv_st[:, wi, :], in_=yvp)
                    else:
                        nc.vector.tensor_copy(out=yv_st[:, wi, :], in_=yvp)
                nc.vector.tensor_copy(out=zv_img[:, ts(wb, WB)], in_=zp)
                eng = [nc.scalar, nc.gpsimd][wb % 2]
                eng.dma_start(out=yv_img_view[:, ts(wb, WB), :], in_=yv_st)

            p2_scores(0)
            p2_scores(1)
            for wb in range(NB2):
                if wb + 2 < NB2:
                    p2_scores(wb + 2)
                p2_consume(wb)

            # transpose Zv image once: [h, w] -> [w, h]
            ztp = psD.tile([96, 96], F32, name="ztp", tag="sp", bufs=2)
            nc.tensor.transpose(ztp, zv_img, identity128[:96, :96])
            nc.vector.tensor_copy(out=zv_T, in_=ztp)

        # ================= Phase 3: horizontal (rows) + combine =================
        x_rows = x.rearrange("(rb r u) c -> rb u r c", r=RB, u=W)
        # y is column-major: pixel (r, u) lives at row u*H + r
        y_rows = y.rearrange("(u rb r) c -> rb u r c", r=RB, rb=H // RB)
        v_rows = v_hbm.rearrange("(rb r u) c -> rb u r c", r=RB, u=W)
        yv_rows = yv_hbm.rearrange("(rb r u) c -> rb u r c", r=RB, u=W)
        with (
            tc.tile_pool(name="p3", bufs=3) as p3,
            tc.tile_pool(name="psE", bufs=3, space="PSUM") as psE,
            tc.tile_pool(name="psF", bufs=2, space="PSUM") as psF,
        ):
            NB3 = H // RB
            e3_blks = {}

            def p3_scores(rb):
                sp3 = psF.tile([96, RB, 96], F32, name="sp3", tag="sp")
                for ri in range(RB):
                    r = rb * RB + ri
                    nc.tensor.matmul(
                        sp3[:, ri, :], lhsT=kT_v[:, r, :], rhs=qT_v[:, r, :]
                    )
                e3_blk = p3.tile([96, RB, 96], BF16, name="e3_blk", bufs=3)
                nc.scalar.activation(
                    out=e3_blk, in_=sp3, func=mybir.ActivationFunctionType.Exp
                )
                e3_blks[rb] = e3_blk

            def p3_consume(rb):
                e3_blk = e3_blks.pop(rb)
                # strided 1KB row-gathers go via gpsimd (SWDGE packs them into
                # 4KB packets); the big xrow f32 reads and y stores alternate
                # on the two HWDGE queues. Deep bufs let xrow/vrow prefetch
                # during earlier phases' DMA slack.
                vrow = p3.tile([96, RB, C], BF16, name="vrow", bufs=6)
                nc.gpsimd.dma_start(out=vrow, in_=v_rows[rb])
                yvrow = p3.tile([96, RB, C], BF16, name="yvrow", bufs=3)
                nc.gpsimd.dma_start(out=yvrow, in_=yv_rows[rb])
                xrow = p3.tile([96, RB, C], F32, name="xrow", bufs=6)
                eng_x = [nc.sync, nc.scalar][rb % 2]
                eng_x.dma_start(out=xrow, in_=x_rows[rb])
                # Z for the whole block, then batched gamma/(Zv+Zh)
                zp3 = psF.tile([96, RB], F32, name="zp3", tag="zp")
                for ri in range(RB):
                    nc.tensor.matmul(
                        zp3[:, ri : ri + 1], lhsT=e3_blk[:, ri, :], rhs=ones96
                    )
                rzg_blk = rzg_all[:, ts(rb, RB)]
                nc.vector.tensor_add(
                    out=rzg_blk, in0=zp3, in1=zv_T[:, ts(rb, RB)]
                )
                nc.vector.reciprocal(out=rzg_blk, in_=rzg_blk)
                nc.vector.tensor_scalar_mul(
                    out=rzg_blk, in0=rzg_blk, scalar1=gamma_sb[:96, :]
                )
                yrow_st = p3.tile([96, RB, C], BF16, name="yrow_st", bufs=3)
                for ri in range(RB):
                    r = rb * RB + ri
                    yp = psE.tile([96, C], F32, name="yp", tag="mm")
                    nc.tensor.matmul(yp, lhsT=e3_blk[:, ri, :], rhs=vrow[:, ri, :],
                                     start=True, stop=False)
                    nc.tensor.matmul(yp, lhsT=id96b, rhs=yvrow[:, ri, :],
                                     start=False, stop=True)
                    nc.vector.scalar_tensor_tensor(
                        out=yrow_st[:, ri, :],
                        in0=yp,
                        scalar=rzg_all[:, ds(r, 1)],
                        in1=xrow[:, ri, :],
                        op0=mult,
                        op1=add,
                    )
                eng_y = [nc.scalar, nc.sync][rb % 2]
                eng_y.dma_start(out=y_rows[rb], in_=yrow_st)

            p3_scores(0)
            for rb in range(NB3):
                if rb + 1 < NB3:
                    p3_scores(rb + 1)
                p3_consume(rb)


_NC_CACHE = None


def _get_nc():
    global _NC_CACHE
    if _NC_CACHE is None:
        _NC_CACHE = build_nc()
    return _NC_CACHE


def run(inputs: dict, trace: bool = False):
    """Run on 8 cores; returns (full_output [8,96,96,512] f32, BassKernelResults)."""
    x = np.ascontiguousarray(np.asarray(inputs["x"], dtype=np.float32))
    B = x.shape[0]
    assert x.shape == (N_CORES, H, W, C), x.shape
    common = {
        "Wq": np.ascontiguousarray(np.asarray(inputs["Wq"], np.float32)),
        "bq": np.ascontiguousarray(np.asarray(inputs["bq"], np.float32)),
        "Wk": np.ascontiguousarray(np.asarray(inputs["Wk"], np.float32)),
        "bk": np.ascontiguousarray(np.asarray(inputs["bk"], np.float32)),
        "Wv": np.ascontiguousarray(np.asarray(inputs["Wv"], np.float32)),
        "bv": np.ascontiguousarray(np.asarray(inputs["bv"], np.float32)),
        "gamma": np.asarray(inputs["gamma"], np.float32).reshape(1, 1).copy(),
    }
    in_maps = [
        {"x": x[b].reshape(NPIX, C), **common} for b in range(B)
    ]
    nc = _get_nc()
    res = run_bass_kernel_spmd(
        nc, in_maps, core_ids=list(range(N_CORES)), trace=trace
    )
    out = np.stack(
        [
            np.asarray(res.results[b]["y"], dtype=np.float32)
            .reshape(W, H, C)
            .transpose(1, 0, 2)
            for b in range(B)
        ],
        axis=0,
    )
    return out, res


def kernel(**inputs) -> np.ndarray:
    out, _ = run(inputs, trace=False)
    return out


if __name__ == "__main__":
    nc = build_nc()
    print("built ok")
